# revision 8
# baseline (speedup 1.0000x reference)
"""Trainium2 Bass kernel for a dense transformer block (attention + GELU MLP).

Strategy (8 cores, SPMD single program):
  - Token/data parallel: core c handles batch b=c//2, token half hh=c%2
    (1024 of 2048 tokens). All weights replicated.
  - K/V are computed for the FULL batch on both cores of a pair
    (duplicated compute, zero communication), Q only for own tokens.
  - Everything on-device is kept feature-major (activations stored
    transposed, [features, tokens]) so every matmul consumes weights in
    their natural [C_in, C_out] layout and no transposes are needed:
        y^T = lhsT.T @ rhs  with lhsT = W, rhs = x^T.
  - Attention: scoresT[k, q] = K_h @ Q_h^T via lhsT=K_h^T slices;
    softmax without max-subtraction (logits ~ N(0,1), fp32 exp is safe);
    causal mask is a multiplicative 0/1 input applied post-exp;
    attn^T = V_h^T @ expT via lhsT=V_h (token-major V); denominators via
    an all-ones stationary matmul accumulated per k-tile on the PE
    (which is otherwise underutilized during attention, and whose
    redundant 128-row output doubles as a partition broadcast).
  - LayerNorm over the partition (feature) axis: sums via ones-matmuls,
    E[x^2] via ACT Square, normalize on DVE, final scale/bias on ACT.
  - Matmuls run in float32r (full PE rate at N>=512, ~1e-4 rounding);
    the FF2 h-path uses bf16 to halve its SBUF/DMA footprint.
  - Big activations are allocated as per-128-row tiles (not one big
    tile) so cross-phase dependencies stay fine-grained.
"""

import sys

sys.path.insert(0, "/opt/trn_rl_repo")

import math
import numpy as np

import concourse.bass as bass
import concourse.bacc as bacc
import concourse.mybir as mybir
import concourse.tile as tile
from concourse import bass_utils

N_CORES = 8
B, T, C = 4, 2048, 2048
NH, D = 16, 128
HF = 4 * C
TL = T // 2        # own tokens per core
TB = T             # batch tokens (K/V extent)
NKT = C // 128     # 16 feature tiles
NHT = HF // 128    # 64 FFN hidden tiles
LN_EPS = 1e-5
SCALE = 1.0 / math.sqrt(D)

f32 = mybir.dt.float32
f32r = mybir.dt.float32r
bf16 = mybir.dt.bfloat16
AF = mybir.ActivationFunctionType
ALU = mybir.AluOpType

_CACHE = {}


def _bias_ap(t, n):
    # [n*128] dram tensor viewed as [128, n] (partition = within-tile index)
    return bass.AP(t, 0, [[1, 128], [128, n]])


def build_nc():
    nc = bacc.Bacc("TRN2", target_bir_lowering=False, debug=False,
                   num_devices=N_CORES)

    # ---- I/O ----
    xT_own = nc.dram_tensor("xT_own", [C, TL], f32r, kind="ExternalInput")
    xT_full = nc.dram_tensor("xT_full", [C, TB], f32r, kind="ExternalInput")
    iota_q = nc.dram_tensor("iota_q", [128, TL], f32, kind="ExternalInput")
    thr_all = nc.dram_tensor("thr_all", [128, NKT], f32, kind="ExternalInput")
    W_qkv = nc.dram_tensor("W_qkv", [C, 3 * C], f32r, kind="ExternalInput")
    b_qs = nc.dram_tensor("b_qs", [C], f32, kind="ExternalInput")
    b_k = nc.dram_tensor("b_k", [C], f32, kind="ExternalInput")
    b_v = nc.dram_tensor("b_v", [C], f32, kind="ExternalInput")
    W_ap = nc.dram_tensor("W_ap", [C, C], f32r, kind="ExternalInput")
    b_ap = nc.dram_tensor("b_ap", [C], f32, kind="ExternalInput")
    ln1_g = nc.dram_tensor("ln1_g", [C], f32, kind="ExternalInput")
    ln1_b = nc.dram_tensor("ln1_b", [C], f32, kind="ExternalInput")
    W_ff = nc.dram_tensor("W_ff", [C, HF], f32r, kind="ExternalInput")
    b_ff = nc.dram_tensor("b_ff", [HF], f32, kind="ExternalInput")
    W_ffp = nc.dram_tensor("W_ffp", [HF, C], bf16, kind="ExternalInput")
    b_ffp = nc.dram_tensor("b_ffp", [C], f32, kind="ExternalInput")
    ln2_g = nc.dram_tensor("ln2_g", [C], f32, kind="ExternalInput")
    ln2_b = nc.dram_tensor("ln2_b", [C], f32, kind="ExternalInput")
    outT = nc.dram_tensor("outT", [C, TL], f32, kind="ExternalOutput")

    with tile.TileContext(nc) as tc:
        with tc.tile_pool(name="dram", bufs=1, space="DRAM") as dram, \
             tc.tile_pool(name="consts", bufs=1) as cp:
            qT = dram.tile([C, TL], f32r)
            kT = dram.tile([C, TB], f32r)
            vTok = dram.tile([TB, C], f32r)

            # constants: biases as [128, n] tiles + all-ones stationary
            bqs_sb = cp.tile([128, NKT], f32)
            nc.sync.dma_start(bqs_sb[:], _bias_ap(b_qs, NKT))
            bk_sb = cp.tile([128, NKT], f32)
            nc.sync.dma_start(bk_sb[:], _bias_ap(b_k, NKT))
            bv_sb = cp.tile([128, NKT], f32)
            nc.sync.dma_start(bv_sb[:], _bias_ap(b_v, NKT))
            bap_sb = cp.tile([128, NKT], f32)
            nc.sync.dma_start(bap_sb[:], _bias_ap(b_ap, NKT))
            g1_sb = cp.tile([128, NKT], f32)
            nc.sync.dma_start(g1_sb[:], _bias_ap(ln1_g, NKT))
            be1_sb = cp.tile([128, NKT], f32)
            nc.sync.dma_start(be1_sb[:], _bias_ap(ln1_b, NKT))
            bff_sb = cp.tile([128, NHT], f32)
            nc.sync.dma_start(bff_sb[:], _bias_ap(b_ff, NHT))
            bffp_sb = cp.tile([128, NKT], f32)
            nc.sync.dma_start(bffp_sb[:], _bias_ap(b_ffp, NKT))
            g2_sb = cp.tile([128, NKT], f32)
            nc.sync.dma_start(g2_sb[:], _bias_ap(ln2_g, NKT))
            be2_sb = cp.tile([128, NKT], f32)
            nc.sync.dma_start(be2_sb[:], _bias_ap(ln2_b, NKT))
            ones_f = cp.tile([128, 128], f32)
            nc.vector.memset(ones_f[:], 1.0)
            ones_r = cp.tile([128, 128], f32r)
            nc.scalar.copy(ones_r[:], ones_f[:])
            eps_sb = cp.tile([128, 1], f32)
            nc.vector.memset(eps_sb[:], LN_EPS)
            iota_sb = cp.tile([128, TL], f32)
            nc.sync.dma_start(iota_sb[:], iota_q.ap())
            thr_sb = cp.tile([128, NKT], f32)
            nc.sync.dma_start(thr_sb[:], thr_all.ap())

            # Q^T = (W_q.T @ xT_own)*s + bqs  -> qT (DRAM)
            with tc.tile_pool(name="xo", bufs=1) as xo_p, \
                 tc.tile_pool(name="wq", bufs=6) as wq_p, \
                 tc.tile_pool(name="qe", bufs=4) as qe_p, \
                 tc.tile_pool(name="psq", bufs=2, space="PSUM") as psq:
                xo = []
                for kt in range(NKT):
                    xot = xo_p.tile([128, TL], f32r, tag=f"xo{kt}",
                                    name=f"xo{kt}")
                    nc.sync.dma_start(
                        xot[:], xT_own.ap()[kt * 128:(kt + 1) * 128, :])
                    xo.append(xot)
                for mg in range(8):           # 2 M-tiles each over Q's 16
                    pss = [psq.tile([128, 512], f32, tag=f"q{i}",
                                    name=f"psq{i}") for i in range(4)]
                    for kt in range(NKT):
                        w2 = wq_p.tile([128, 256], f32r, tag="w")
                        nc.sync.dma_start(
                            w2[:],
                            W_qkv.ap()[kt * 128:(kt + 1) * 128,
                                       mg * 256:(mg + 1) * 256])
                        for mi in range(2):
                            for th in range(2):
                                nc.tensor.matmul(
                                    pss[mi * 2 + th][:],
                                    w2[:, mi * 128:(mi + 1) * 128],
                                    xo[kt][:, th * 512:(th + 1) * 512],
                                    start=(kt == 0), stop=(kt == NKT - 1))
                    for mi in range(2):
                        m = mg * 2 + mi
                        for th in range(2):
                            ev = qe_p.tile([128, 512], f32r, tag="e")
                            nc.vector.tensor_scalar(
                                ev[:], pss[mi * 2 + th][:], SCALE,
                                bqs_sb[:, m:m + 1], ALU.mult, ALU.add)
                            nc.sync.dma_start(
                                qT[m * 128:(m + 1) * 128,
                                   th * 512:(th + 1) * 512], ev[:])

            # ============ Phase V, K (full batch), then Q (own half) ========
            with tc.tile_pool(name="xf", bufs=1) as xf_p, \
                 tc.tile_pool(name="wk", bufs=6) as wk_p, \
                 tc.tile_pool(name="kv_e", bufs=6) as kve_p, \
                 tc.tile_pool(name="psk", bufs=2, space="PSUM") as psk:
                xf = []
                for kt in range(NKT):
                    xft = xf_p.tile([128, TB], f32r, tag=f"xf{kt}",
                                    name=f"xf{kt}")
                    nc.sync.dma_start(
                        xft[:], xT_full.ap()[kt * 128:(kt + 1) * 128, :])
                    xf.append(xft)
                # V token-major [2048, C]
                for nv in range(4):
                    for mtg in range(2):
                        pss = [psk.tile([128, 512], f32, tag=f"k{i % 4}",
                                        name=f"pskv{i}") for i in range(8)]
                        for kt in range(NKT):
                            wv = wk_p.tile([128, 512], f32r, tag="wv")
                            nc.sync.dma_start(
                                wv[:],
                                W_qkv.ap()[kt * 128:(kt + 1) * 128,
                                           2 * C + nv * 512:
                                           2 * C + (nv + 1) * 512])
                            for mt in range(8):
                                tt = mtg * 8 + mt
                                nc.tensor.matmul(
                                    pss[mt][:],
                                    xf[kt][:, tt * 128:(tt + 1) * 128],
                                    wv[:],
                                    start=(kt == 0), stop=(kt == NKT - 1))
                        for mt in range(8):
                            tt = mtg * 8 + mt
                            ev = kve_p.tile([128, 512], f32r, tag="e")
                            nc.vector.tensor_copy(ev[:], pss[mt][:])
                            nc.sync.dma_start(
                                vTok[tt * 128:(tt + 1) * 128,
                                     nv * 512:(nv + 1) * 512], ev[:])
                # K^T [C, 2048]
                for m in range(NKT):
                    pss = [psk.tile([128, 512], f32, tag=f"k{i}",
                                    name=f"psk{i}") for i in range(4)]
                    for kt in range(NKT):
                        w1 = wk_p.tile([128, 128], f32r, tag="w")
                        nc.sync.dma_start(
                            w1[:],
                            W_qkv.ap()[kt * 128:(kt + 1) * 128,
                                       C + m * 128:C + (m + 1) * 128])
                        for tcn in range(4):
                            nc.tensor.matmul(
                                pss[tcn][:], w1[:],
                                xf[kt][:, tcn * 512:(tcn + 1) * 512],
                                start=(kt == 0), stop=(kt == NKT - 1))
                    for tcn in range(4):
                        ev = kve_p.tile([128, 512], f32r, tag="e")
                        nc.vector.tensor_scalar_add(
                            ev[:], pss[tcn][:], bk_sb[:, m:m + 1])
                        nc.sync.dma_start(
                            kT[m * 128:(m + 1) * 128,
                               tcn * 512:(tcn + 1) * 512], ev[:])

            # =================== Attention + projection ===================
            # xres pool opened first so it survives into LN1 (stack alloc).
            with tc.tile_pool(name="xres", bufs=1) as xr_p, \
                 tc.tile_pool(name="attnT", bufs=1) as at_p:
                xres = [xr_p.tile([128, TL], f32r, tag=f"xr{kt}",
                                  name=f"xres{kt}") for kt in range(NKT)]
                attnT = [at_p.tile([128, TL], f32r, tag=f"at{h}",
                                   name=f"attnT{h}") for h in range(NH)]
                with tc.tile_pool(name="kv", bufs=1) as kv_p, \
                     tc.tile_pool(name="qh", bufs=2) as qh_p, \
                     tc.tile_pool(name="et", bufs=3) as et_p, \
                     tc.tile_pool(name="ex", bufs=3) as ex_p, \
                     tc.tile_pool(name="nrm", bufs=1) as nrm_p, \
                     tc.tile_pool(name="dn", bufs=2) as dn_p, \
                     tc.tile_pool(name="ps_s", bufs=2, space="PSUM") as ps_s_p, \
                     tc.tile_pool(name="ps_av", bufs=2, space="PSUM") as ps_av_p:
                    for h in range(NH):
                        kc = []
                        vc = []
                        for cq in range(4):
                            kct = kv_p.tile([128, 512], f32r, tag=f"kc{cq}",
                                            name=f"kc{cq}")
                            nc.sync.dma_start(
                                kct[:], kT[h * 128:(h + 1) * 128,
                                           cq * 512:(cq + 1) * 512])
                            kc.append(kct)
                            vct = kv_p.tile([128, 512], f32r, tag=f"vc{cq}",
                                            name=f"vc{cq}")
                            nc.sync.dma_start(
                                vct[:],
                                bass.AP(vTok.tensor,
                                        h * 128 + cq * 4 * 128 * C,
                                        [[C, 128], [128 * C, 4], [1, 128]]))
                            vc.append(vct)
                        q_sb = qh_p.tile([128, TL], f32r, tag="q")
                        nc.sync.dma_start(
                            q_sb[:], qT[h * 128:(h + 1) * 128, :])
                        ps_av = ps_av_p.tile([128, TL], f32, tag="av",
                                             name="ps_av")
                        dacc = dn_p.tile([128, TL], f32r, tag="da",
                                         name="dacc")
                        for kt in range(NKT):
                            ps_s = ps_s_p.tile([128, TL], f32, tag="s",
                                               name="ps_s")
                            for th in range(2):
                                nc.tensor.matmul(
                                    ps_s[:, th * 512:(th + 1) * 512],
                                    kc[kt // 4][:, (kt % 4) * 128:
                                                (kt % 4 + 1) * 128],
                                    q_sb[:, th * 512:(th + 1) * 512],
                                    start=True, stop=True)
                            et = et_p.tile([128, TL], f32, tag="et",
                                           name="et")
                            nc.scalar.activation(et[:], ps_s[:], AF.Exp)
                            e_kt = ex_p.tile([128, TL], f32r, tag="ex",
                                             name="e_kt")
                            nc.vector.scalar_tensor_tensor(
                                e_kt[:], iota_sb[:], thr_sb[:, kt:kt + 1],
                                et[:], ALU.is_ge, ALU.mult)
                            if kt == 0:
                                nc.vector.tensor_copy(dacc[:], e_kt[:])
                            else:
                                nc.vector.tensor_add(dacc[:], dacc[:], e_kt[:])
                            for th in range(2):
                                sl = slice(th * 512, (th + 1) * 512)
                                nc.tensor.matmul(
                                    ps_av[:, sl],
                                    vc[kt // 4][:, (kt % 4) * 128:
                                                (kt % 4 + 1) * 128],
                                    e_kt[:, sl],
                                    start=(kt == 0), stop=(kt == NKT - 1))
                        ps_dn = ps_s_p.tile([128, TL], f32, tag="s",
                                            name="ps_dn")
                        for th in range(2):
                            sl = slice(th * 512, (th + 1) * 512)
                            nc.tensor.matmul(ps_dn[:, sl], ones_r[:],
                                             dacc[:, sl],
                                             start=True, stop=True)
                        rec = nrm_p.tile([128, TL], f32, tag="rec",
                                         name="rec")
                        nc.vector.reciprocal(rec[:], ps_dn[:])
                        avn = nrm_p.tile([128, TL], f32, tag="avn",
                                         name="avn")
                        nc.vector.tensor_mul(avn[:], ps_av[:], rec[:])
                        nc.scalar.activation(
                            attnT[h][:], avn[:],
                            AF.Identity, bias=bv_sb[:, h:h + 1])

                # ---- attention projection + residual -> xres (SBUF) ----
                with tc.tile_pool(name="wp", bufs=6) as wp_p, \
                     tc.tile_pool(name="xos", bufs=3) as xos_p, \
                     tc.tile_pool(name="psp", bufs=2, space="PSUM") as psp:
                    for mg in range(8):
                        pss = [psp.tile([128, 512], f32, tag=f"p{i}",
                                        name=f"psp{i}") for i in range(4)]
                        for kt in range(NKT):
                            w2 = wp_p.tile([128, 256], f32r, tag="w")
                            nc.sync.dma_start(
                                w2[:],
                                W_ap.ap()[kt * 128:(kt + 1) * 128,
                                          mg * 256:(mg + 1) * 256])
                            for mi in range(2):
                                for th in range(2):
                                    nc.tensor.matmul(
                                        pss[mi * 2 + th][:],
                                        w2[:, mi * 128:(mi + 1) * 128],
                                        attnT[kt][:, th * 512:(th + 1) * 512],
                                        start=(kt == 0), stop=(kt == NKT - 1))
                        for mi in range(2):
                            m = mg * 2 + mi
                            xot = xos_p.tile([128, TL], f32r, tag="x",
                                             name="xot")
                            nc.sync.dma_start(
                                xot[:], xT_own.ap()[m * 128:(m + 1) * 128, :])
                            for th in range(2):
                                nc.vector.scalar_tensor_tensor(
                                    xres[m][:, th * 512:(th + 1) * 512],
                                    pss[mi * 2 + th][:],
                                    bap_sb[:, m:m + 1],
                                    xot[:, th * 512:(th + 1) * 512],
                                    ALU.add, ALU.add)

            # ============== LN1 -> x1 ; FFN ; LN2 -> out ==============
            with tc.tile_pool(name="x1", bufs=1) as x1_p:
                x1 = [x1_p.tile([128, TL], f32r, tag=f"x1{kt}",
                                name=f"x1t{kt}") for kt in range(NKT)]

                def layernorm(src, emit_out, psln, tmp_p):
                    # src: list of NKT [128, TL] f32r tiles (feature-major)
                    ps_sum = psln.tile([128, TL], f32, tag="su", name="ps_sum")
                    ps_sq = psln.tile([128, TL], f32, tag="sq", name="ps_sq")
                    for kt in range(NKT):
                        sq = tmp_p.tile([128, TL], f32r, tag="sqt", name="sq")
                        nc.scalar.activation(sq[:], src[kt][:], AF.Square)
                        for th in range(2):
                            sl = slice(th * 512, (th + 1) * 512)
                            nc.tensor.matmul(
                                ps_sum[:, sl], ones_r[:], src[kt][:, sl],
                                start=(kt == 0), stop=(kt == NKT - 1))
                            nc.tensor.matmul(
                                ps_sq[:, sl], ones_r[:], sq[:, sl],
                                start=(kt == 0), stop=(kt == NKT - 1))
                    mu = tmp_p.tile([128, TL], f32, tag="mu", name="mu")
                    nc.vector.tensor_scalar_mul(mu[:], ps_sum[:], 1.0 / C)
                    msq = tmp_p.tile([128, TL], f32, tag="msq", name="msq")
                    nc.vector.tensor_scalar_mul(msq[:], ps_sq[:], 1.0 / C)
                    mu2 = tmp_p.tile([128, TL], f32, tag="mu2", name="mu2")
                    nc.vector.tensor_mul(mu2[:], mu[:], mu[:])
                    var = tmp_p.tile([128, TL], f32, tag="var", name="var")
                    nc.vector.tensor_sub(var[:], msq[:], mu2[:])
                    sd = tmp_p.tile([128, TL], f32, tag="sd", name="sd")
                    nc.scalar.activation(sd[:], var[:], AF.Sqrt, bias=eps_sb[:])
                    rstd = tmp_p.tile([128, TL], f32, tag="rstd", name="rstd")
                    nc.vector.reciprocal(rstd[:], sd[:])
                    for kt in range(NKT):
                        t1 = tmp_p.tile([128, TL], f32, tag="t1", name="t1")
                        nc.vector.tensor_sub(t1[:], src[kt][:], mu[:])
                        t2 = tmp_p.tile([128, TL], f32, tag="t2", name="t2")
                        nc.vector.tensor_mul(t2[:], t1[:], rstd[:])
                        emit_out(kt, t2)

                # ---- LN1 ----
                with tc.tile_pool(name="lt1", bufs=1) as lt1_p, \
                     tc.tile_pool(name="psl1", bufs=1, space="PSUM") as psl1:
                    def emit_x1(kt, t2):
                        nc.scalar.activation(
                            x1[kt][:], t2[:], AF.Identity,
                            bias=be1_sb[:, kt:kt + 1],
                            scale=g1_sb[:, kt:kt + 1])

                    layernorm(xres, emit_x1, psl1, lt1_p)

                # ---- FFN ----
                with tc.tile_pool(name="hb", bufs=1) as hb_p, \
                     tc.tile_pool(name="wf", bufs=4) as wf_p, \
                     tc.tile_pool(name="wfp", bufs=6) as wfp_p, \
                     tc.tile_pool(name="ps_h", bufs=1, space="PSUM") as ps_h_p, \
                     tc.tile_pool(name="ps_y", bufs=1, space="PSUM") as ps_y_p:
                    for th in range(2):
                        hs = [hb_p.tile([128, 512], bf16, tag=f"h{mh}",
                                        name=f"hs{mh}") for mh in range(NHT)]
                        # FF1: h = gelu(W_ff.T @ x1 + b_ff), half of T
                        for mtg in range(NHT // 4):
                            pss = [ps_h_p.tile([128, 512], f32, tag=f"h{i}",
                                               name=f"psh{i}")
                                   for i in range(4)]
                            for kt in range(NKT):
                                wf = wf_p.tile([128, 512], f32r, tag="w")
                                nc.sync.dma_start(
                                    wf[:],
                                    W_ff.ap()[kt * 128:(kt + 1) * 128,
                                              mtg * 512:(mtg + 1) * 512])
                                for mt in range(4):
                                    nc.tensor.matmul(
                                        pss[mt][:],
                                        wf[:, mt * 128:(mt + 1) * 128],
                                        x1[kt][:, th * 512:(th + 1) * 512],
                                        start=(kt == 0), stop=(kt == NKT - 1))
                            for mt in range(4):
                                mh = mtg * 4 + mt
                                nc.scalar.activation(
                                    hs[mh][:], pss[mt][:], AF.Gelu,
                                    bias=bff_sb[:, mh:mh + 1])
                        # FF2: x2 = x1 + W_ffp.T @ h + b_ffp (into x1)
                        for mg in range(4):
                            pss = [ps_y_p.tile([128, 512], f32, tag=f"y{i}",
                                               name=f"psy{i}")
                                   for i in range(4)]
                            for kt in range(NHT):
                                wfp = wfp_p.tile([128, 512], bf16, tag="w")
                                nc.sync.dma_start(
                                    wfp[:],
                                    W_ffp.ap()[kt * 128:(kt + 1) * 128,
                                               mg * 512:(mg + 1) * 512])
                                for mi in range(4):
                                    nc.tensor.matmul(
                                        pss[mi][:],
                                        wfp[:, mi * 128:(mi + 1) * 128],
                                        hs[kt][:],
                                        start=(kt == 0), stop=(kt == NHT - 1))
                            for mi in range(4):
                                m = mg * 4 + mi
                                sl = slice(th * 512, (th + 1) * 512)
                                nc.vector.scalar_tensor_tensor(
                                    x1[m][:, sl], pss[mi][:],
                                    bffp_sb[:, m:m + 1], x1[m][:, sl],
                                    ALU.add, ALU.add)

                # ---- LN2 -> output ----
                with tc.tile_pool(name="lt2", bufs=1) as lt2_p, \
                     tc.tile_pool(name="ob", bufs=3) as ob_p, \
                     tc.tile_pool(name="psl2", bufs=1, space="PSUM") as psl2:
                    def emit_out(kt, t2):
                        ot = ob_p.tile([128, TL], f32, tag="o", name="ot")
                        nc.scalar.activation(
                            ot[:], t2[:], AF.Identity,
                            bias=be2_sb[:, kt:kt + 1],
                            scale=g2_sb[:, kt:kt + 1])
                        nc.sync.dma_start(
                            outT.ap()[kt * 128:(kt + 1) * 128, :], ot[:])

                    layernorm(x1, emit_out, psl2, lt2_p)

    nc.compile()
    return nc


def _get_nc():
    if "nc" not in _CACHE:
        _CACHE["nc"] = build_nc()
    return _CACHE["nc"]


def _prepare_in_maps(inputs):
    x = np.asarray(inputs["x"], dtype=np.float32)
    W_qkv = np.asarray(inputs["W_qkv"], dtype=np.float32)
    b_qkv = np.asarray(inputs["b_qkv"], dtype=np.float32)
    W_ap = np.asarray(inputs["W_attn_proj"], dtype=np.float32)
    b_ap = np.asarray(inputs["b_attn_proj"], dtype=np.float32)
    ln1_g = np.asarray(inputs["ln1_g"], dtype=np.float32)
    ln1_b = np.asarray(inputs["ln1_b"], dtype=np.float32)
    W_ff = np.asarray(inputs["W_ff"], dtype=np.float32)
    b_ff = np.asarray(inputs["b_ff"], dtype=np.float32)
    W_ffp = np.asarray(inputs["W_ff_proj"], dtype=np.float32)
    b_ffp = np.asarray(inputs["b_ff_proj"], dtype=np.float32)
    ln2_g = np.asarray(inputs["ln2_g"], dtype=np.float32)
    ln2_b = np.asarray(inputs["ln2_b"], dtype=np.float32)

    import ml_dtypes
    W_ffp_bf = W_ffp.astype(ml_dtypes.bfloat16)

    b_qs = np.ascontiguousarray(b_qkv[:C] * SCALE)
    b_kk = np.ascontiguousarray(b_qkv[C:2 * C])
    b_vv = np.ascontiguousarray(b_qkv[2 * C:])

    iota_np = np.broadcast_to(np.arange(TL, dtype=np.float32),
                              (128, TL)).copy()
    p_idx = np.arange(128, dtype=np.float32)[:, None]
    kt_idx = np.arange(NKT, dtype=np.float32)[None, :] * 128
    thrs = [np.ascontiguousarray(p_idx + kt_idx - hh * TL)
            for hh in range(2)]

    xT_fulls = [np.ascontiguousarray(x[b].T) for b in range(B)]

    common = dict(W_qkv=W_qkv, b_qs=b_qs, b_k=b_kk, b_v=b_vv, W_ap=W_ap,
                  b_ap=b_ap, ln1_g=ln1_g, ln1_b=ln1_b, W_ff=W_ff, b_ff=b_ff,
                  W_ffp=W_ffp_bf, b_ffp=b_ffp, ln2_g=ln2_g, ln2_b=ln2_b)
    in_maps = []
    for c in range(N_CORES):
        b, hh = c // 2, c % 2
        m = dict(common)
        m["xT_full"] = xT_fulls[b]
        m["xT_own"] = np.ascontiguousarray(
            xT_fulls[b][:, hh * TL:(hh + 1) * TL])
        m["iota_q"] = iota_np
        m["thr_all"] = thrs[hh]
        in_maps.append(m)
    return in_maps


def run_kernel(inputs, trace=False):
    nc = _get_nc()
    in_maps = _prepare_in_maps(inputs)
    res = bass_utils.run_bass_kernel_spmd(
        nc, in_maps, core_ids=list(range(N_CORES)), trace=trace)
    out = np.empty((B, T, C), dtype=np.float32)
    for c in range(N_CORES):
        b, hh = c // 2, c % 2
        out[b, hh * TL:(hh + 1) * TL, :] = res.results[c]["outT"].T
    return out, res


def kernel(**inputs) -> np.ndarray:
    out, _ = run_kernel(inputs, trace=False)
    return out


# revision 9
# speedup vs baseline: 1.0401x; 1.0401x over previous
"""Trainium2 Bass kernel for a dense transformer block (attention + GELU MLP).

Strategy (8 cores, SPMD single program):
  - Token/data parallel: core c handles batch b=c//2, token half hh=c%2
    (1024 of 2048 tokens). All weights replicated.
  - K/V are computed for the FULL batch on both cores of a pair
    (duplicated compute, zero communication), Q only for own tokens.
  - Everything on-device is kept feature-major (activations stored
    transposed, [features, tokens]) so every matmul consumes weights in
    their natural [C_in, C_out] layout and no transposes are needed:
        y^T = lhsT.T @ rhs  with lhsT = W, rhs = x^T.
  - Attention: scoresT[k, q] = K_h @ Q_h^T via lhsT=K_h^T slices;
    softmax without max-subtraction (logits ~ N(0,1), fp32 exp is safe);
    causal mask is a multiplicative 0/1 input applied post-exp;
    attn^T = V_h^T @ expT via lhsT=V_h (token-major V); denominators via
    an all-ones stationary matmul accumulated per k-tile on the PE
    (which is otherwise underutilized during attention, and whose
    redundant 128-row output doubles as a partition broadcast).
  - LayerNorm over the partition (feature) axis: sums via ones-matmuls,
    E[x^2] via ACT Square, normalize on DVE, final scale/bias on ACT.
  - Matmuls run in float32r (full PE rate at N>=512, ~1e-4 rounding);
    the FF2 h-path uses bf16 to halve its SBUF/DMA footprint.
  - Big activations are allocated as per-128-row tiles (not one big
    tile) so cross-phase dependencies stay fine-grained.
"""

import sys

sys.path.insert(0, "/opt/trn_rl_repo")

import math
import numpy as np

import concourse.bass as bass
import concourse.bacc as bacc
import concourse.mybir as mybir
import concourse.tile as tile
from concourse import bass_utils

N_CORES = 8
B, T, C = 4, 2048, 2048
NH, D = 16, 128
HF = 4 * C
TL = T // 2        # own tokens per core
TB = T             # batch tokens (K/V extent)
NKT = C // 128     # 16 feature tiles
NHT = HF // 128    # 64 FFN hidden tiles
LN_EPS = 1e-5
SCALE = 1.0 / math.sqrt(D)

f32 = mybir.dt.float32
f32r = mybir.dt.float32r
bf16 = mybir.dt.bfloat16
AF = mybir.ActivationFunctionType
ALU = mybir.AluOpType

_CACHE = {}


def _bias_ap(t, n):
    # [n*128] dram tensor viewed as [128, n] (partition = within-tile index)
    return bass.AP(t, 0, [[1, 128], [128, n]])


def build_nc():
    nc = bacc.Bacc("TRN2", target_bir_lowering=False, debug=False,
                   num_devices=N_CORES)

    # ---- I/O ----
    xT_own = nc.dram_tensor("xT_own", [C, TL], f32r, kind="ExternalInput")
    xT_full = nc.dram_tensor("xT_full", [C, TB], f32r, kind="ExternalInput")
    iota_q = nc.dram_tensor("iota_q", [128, TL], f32, kind="ExternalInput")
    thr_all = nc.dram_tensor("thr_all", [128, NKT], f32, kind="ExternalInput")
    W_qkv = nc.dram_tensor("W_qkv", [C, 3 * C], f32r, kind="ExternalInput")
    b_qs = nc.dram_tensor("b_qs", [C], f32, kind="ExternalInput")
    b_k = nc.dram_tensor("b_k", [C], f32, kind="ExternalInput")
    b_v = nc.dram_tensor("b_v", [C], f32, kind="ExternalInput")
    W_ap = nc.dram_tensor("W_ap", [C, C], f32r, kind="ExternalInput")
    b_ap = nc.dram_tensor("b_ap", [C], f32, kind="ExternalInput")
    ln1_g = nc.dram_tensor("ln1_g", [C], f32, kind="ExternalInput")
    ln1_b = nc.dram_tensor("ln1_b", [C], f32, kind="ExternalInput")
    W_ff = nc.dram_tensor("W_ff", [C, HF], f32r, kind="ExternalInput")
    b_ff = nc.dram_tensor("b_ff", [HF], f32, kind="ExternalInput")
    W_ffp = nc.dram_tensor("W_ffp", [HF, C], bf16, kind="ExternalInput")
    b_ffp = nc.dram_tensor("b_ffp", [C], f32, kind="ExternalInput")
    ln2_g = nc.dram_tensor("ln2_g", [C], f32, kind="ExternalInput")
    ln2_b = nc.dram_tensor("ln2_b", [C], f32, kind="ExternalInput")
    outT = nc.dram_tensor("outT", [C, TL], f32, kind="ExternalOutput")

    with tile.TileContext(nc) as tc:
        with tc.tile_pool(name="dram", bufs=1, space="DRAM") as dram, \
             tc.tile_pool(name="consts", bufs=1) as cp:
            qT = dram.tile([C, TL], f32r)
            kT = dram.tile([C, TB], f32r)
            vTok = dram.tile([TB, C], f32r)

            # constants: biases as [128, n] tiles + all-ones stationary
            bqs_sb = cp.tile([128, NKT], f32)
            nc.sync.dma_start(bqs_sb[:], _bias_ap(b_qs, NKT))
            bk_sb = cp.tile([128, NKT], f32)
            nc.sync.dma_start(bk_sb[:], _bias_ap(b_k, NKT))
            bv_sb = cp.tile([128, NKT], f32)
            nc.sync.dma_start(bv_sb[:], _bias_ap(b_v, NKT))
            bap_sb = cp.tile([128, NKT], f32)
            nc.sync.dma_start(bap_sb[:], _bias_ap(b_ap, NKT))
            g1_sb = cp.tile([128, NKT], f32)
            nc.sync.dma_start(g1_sb[:], _bias_ap(ln1_g, NKT))
            be1_sb = cp.tile([128, NKT], f32)
            nc.sync.dma_start(be1_sb[:], _bias_ap(ln1_b, NKT))
            bff_sb = cp.tile([128, NHT], f32)
            nc.sync.dma_start(bff_sb[:], _bias_ap(b_ff, NHT))
            bffp_sb = cp.tile([128, NKT], f32)
            nc.sync.dma_start(bffp_sb[:], _bias_ap(b_ffp, NKT))
            g2_sb = cp.tile([128, NKT], f32)
            nc.sync.dma_start(g2_sb[:], _bias_ap(ln2_g, NKT))
            be2_sb = cp.tile([128, NKT], f32)
            nc.sync.dma_start(be2_sb[:], _bias_ap(ln2_b, NKT))
            ones_f = cp.tile([128, 128], f32)
            nc.vector.memset(ones_f[:], 1.0)
            ones_r = cp.tile([128, 128], f32r)
            nc.scalar.copy(ones_r[:], ones_f[:])
            eps_sb = cp.tile([128, 1], f32)
            nc.vector.memset(eps_sb[:], LN_EPS)
            iota_sb = cp.tile([128, TL], f32)
            nc.sync.dma_start(iota_sb[:], iota_q.ap())
            thr_sb = cp.tile([128, NKT], f32)
            nc.sync.dma_start(thr_sb[:], thr_all.ap())

            # ============ Phase V, K (full batch), then Q (own half) ========
            with tc.tile_pool(name="xf", bufs=1) as xf_p, \
                 tc.tile_pool(name="wk", bufs=6) as wk_p, \
                 tc.tile_pool(name="kv_e", bufs=6) as kve_p, \
                 tc.tile_pool(name="psk", bufs=2, space="PSUM") as psk:
                xf = []
                for kt in range(NKT):
                    xft = xf_p.tile([128, TB], f32r, tag=f"xf{kt}",
                                    name=f"xf{kt}")
                    nc.sync.dma_start(
                        xft[:], xT_full.ap()[kt * 128:(kt + 1) * 128, :])
                    xf.append(xft)
                # V token-major [2048, C]
                for nv in range(4):
                    for mtg in range(2):
                        pss = [psk.tile([128, 512], f32, tag=f"k{i % 4}",
                                        name=f"pskv{i}") for i in range(8)]
                        for kt in range(NKT):
                            wv = wk_p.tile([128, 512], f32r, tag="wv")
                            nc.sync.dma_start(
                                wv[:],
                                W_qkv.ap()[kt * 128:(kt + 1) * 128,
                                           2 * C + nv * 512:
                                           2 * C + (nv + 1) * 512])
                            for mt in range(8):
                                tt = mtg * 8 + mt
                                nc.tensor.matmul(
                                    pss[mt][:],
                                    xf[kt][:, tt * 128:(tt + 1) * 128],
                                    wv[:],
                                    start=(kt == 0), stop=(kt == NKT - 1))
                        for mt in range(8):
                            tt = mtg * 8 + mt
                            ev = kve_p.tile([128, 512], f32r, tag="e")
                            nc.vector.tensor_copy(ev[:], pss[mt][:])
                            nc.sync.dma_start(
                                vTok[tt * 128:(tt + 1) * 128,
                                     nv * 512:(nv + 1) * 512], ev[:])
                # K^T [C, 2048]
                for m in range(NKT):
                    pss = [psk.tile([128, 512], f32, tag=f"k{i}",
                                    name=f"psk{i}") for i in range(4)]
                    for kt in range(NKT):
                        w1 = wk_p.tile([128, 128], f32r, tag="w")
                        nc.sync.dma_start(
                            w1[:],
                            W_qkv.ap()[kt * 128:(kt + 1) * 128,
                                       C + m * 128:C + (m + 1) * 128])
                        for tcn in range(4):
                            nc.tensor.matmul(
                                pss[tcn][:], w1[:],
                                xf[kt][:, tcn * 512:(tcn + 1) * 512],
                                start=(kt == 0), stop=(kt == NKT - 1))
                    for tcn in range(4):
                        ev = kve_p.tile([128, 512], f32r, tag="e")
                        nc.vector.tensor_scalar_add(
                            ev[:], pss[tcn][:], bk_sb[:, m:m + 1])
                        nc.sync.dma_start(
                            kT[m * 128:(m + 1) * 128,
                               tcn * 512:(tcn + 1) * 512], ev[:])

            # Q^T = (W_q.T @ xT_own)*s + bqs  -> qT (DRAM)
            with tc.tile_pool(name="xo", bufs=1) as xo_p, \
                 tc.tile_pool(name="wq", bufs=6) as wq_p, \
                 tc.tile_pool(name="qe", bufs=4) as qe_p, \
                 tc.tile_pool(name="psq", bufs=2, space="PSUM") as psq:
                xo = []
                for kt in range(NKT):
                    xot = xo_p.tile([128, TL], f32r, tag=f"xo{kt}",
                                    name=f"xo{kt}")
                    nc.sync.dma_start(
                        xot[:], xT_own.ap()[kt * 128:(kt + 1) * 128, :])
                    xo.append(xot)
                for mg in range(8):           # 2 M-tiles each over Q's 16
                    pss = [psq.tile([128, 512], f32, tag=f"q{i}",
                                    name=f"psq{i}") for i in range(4)]
                    for kt in range(NKT):
                        w2 = wq_p.tile([128, 256], f32r, tag="w")
                        nc.sync.dma_start(
                            w2[:],
                            W_qkv.ap()[kt * 128:(kt + 1) * 128,
                                       mg * 256:(mg + 1) * 256])
                        for mi in range(2):
                            for th in range(2):
                                nc.tensor.matmul(
                                    pss[mi * 2 + th][:],
                                    w2[:, mi * 128:(mi + 1) * 128],
                                    xo[kt][:, th * 512:(th + 1) * 512],
                                    start=(kt == 0), stop=(kt == NKT - 1))
                    for mi in range(2):
                        m = mg * 2 + mi
                        for th in range(2):
                            ev = qe_p.tile([128, 512], f32r, tag="e")
                            nc.vector.tensor_scalar(
                                ev[:], pss[mi * 2 + th][:], SCALE,
                                bqs_sb[:, m:m + 1], ALU.mult, ALU.add)
                            nc.sync.dma_start(
                                qT[m * 128:(m + 1) * 128,
                                   th * 512:(th + 1) * 512], ev[:])

            # =================== Attention + projection ===================
            # xres pool opened first so it survives into LN1 (stack alloc).
            with tc.tile_pool(name="xres", bufs=1) as xr_p, \
                 tc.tile_pool(name="attnT", bufs=1) as at_p:
                xres = [xr_p.tile([128, TL], f32r, tag=f"xr{kt}",
                                  name=f"xres{kt}") for kt in range(NKT)]
                attnT = [at_p.tile([128, TL], f32r, tag=f"at{h}",
                                   name=f"attnT{h}") for h in range(NH)]
                with tc.tile_pool(name="kv", bufs=1) as kv_p, \
                     tc.tile_pool(name="qh", bufs=2) as qh_p, \
                     tc.tile_pool(name="et", bufs=2) as et_p, \
                     tc.tile_pool(name="ex", bufs=2) as ex_p, \
                     tc.tile_pool(name="nrm", bufs=1) as nrm_p, \
                     tc.tile_pool(name="ps_s", bufs=2, space="PSUM") as ps_s_p, \
                     tc.tile_pool(name="ps_av", bufs=1, space="PSUM") as ps_av_p, \
                     tc.tile_pool(name="ps_d", bufs=1, space="PSUM") as ps_d_p:
                    for h in range(NH):
                        kc = []
                        vc = []
                        for cq in range(4):
                            kct = kv_p.tile([128, 512], f32r, tag=f"kc{cq}",
                                            name=f"kc{cq}")
                            nc.sync.dma_start(
                                kct[:], kT[h * 128:(h + 1) * 128,
                                           cq * 512:(cq + 1) * 512])
                            kc.append(kct)
                            vct = kv_p.tile([128, 512], f32r, tag=f"vc{cq}",
                                            name=f"vc{cq}")
                            nc.sync.dma_start(
                                vct[:],
                                bass.AP(vTok.tensor,
                                        h * 128 + cq * 4 * 128 * C,
                                        [[C, 128], [128 * C, 4], [1, 128]]))
                            vc.append(vct)
                        q_sb = qh_p.tile([128, TL], f32r, tag="q")
                        nc.sync.dma_start(
                            q_sb[:], qT[h * 128:(h + 1) * 128, :])
                        ps_av = ps_av_p.tile([128, TL], f32, tag="av",
                                             name="ps_av")
                        ps_d = ps_d_p.tile([128, TL], f32, tag="dn",
                                           name="ps_d")
                        for kt in range(NKT):
                            ps_s = ps_s_p.tile([128, TL], f32, tag="s",
                                               name="ps_s")
                            for th in range(2):
                                nc.tensor.matmul(
                                    ps_s[:, th * 512:(th + 1) * 512],
                                    kc[kt // 4][:, (kt % 4) * 128:
                                                (kt % 4 + 1) * 128],
                                    q_sb[:, th * 512:(th + 1) * 512],
                                    start=True, stop=True)
                            et = et_p.tile([128, TL], f32, tag="et",
                                           name="et")
                            nc.scalar.activation(et[:], ps_s[:], AF.Exp)
                            e_kt = ex_p.tile([128, TL], f32r, tag="ex",
                                             name="e_kt")
                            nc.vector.scalar_tensor_tensor(
                                e_kt[:], iota_sb[:], thr_sb[:, kt:kt + 1],
                                et[:], ALU.is_ge, ALU.mult)
                            for th in range(2):
                                sl = slice(th * 512, (th + 1) * 512)
                                nc.tensor.matmul(
                                    ps_av[:, sl],
                                    vc[kt // 4][:, (kt % 4) * 128:
                                                (kt % 4 + 1) * 128],
                                    e_kt[:, sl],
                                    start=(kt == 0), stop=(kt == NKT - 1))
                                nc.tensor.matmul(
                                    ps_d[:, sl], ones_r[:], e_kt[:, sl],
                                    start=(kt == 0), stop=(kt == NKT - 1))
                        rec = nrm_p.tile([128, TL], f32, tag="rec",
                                         name="rec")
                        nc.vector.reciprocal(rec[:], ps_d[:])
                        avn = nrm_p.tile([128, TL], f32, tag="avn",
                                         name="avn")
                        nc.vector.tensor_mul(avn[:], ps_av[:], rec[:])
                        nc.scalar.activation(
                            attnT[h][:], avn[:],
                            AF.Identity, bias=bv_sb[:, h:h + 1])

                # ---- attention projection + residual -> xres (SBUF) ----
                with tc.tile_pool(name="wp", bufs=6) as wp_p, \
                     tc.tile_pool(name="xos", bufs=3) as xos_p, \
                     tc.tile_pool(name="psp", bufs=2, space="PSUM") as psp:
                    for mg in range(8):
                        pss = [psp.tile([128, 512], f32, tag=f"p{i}",
                                        name=f"psp{i}") for i in range(4)]
                        for kt in range(NKT):
                            w2 = wp_p.tile([128, 256], f32r, tag="w")
                            nc.sync.dma_start(
                                w2[:],
                                W_ap.ap()[kt * 128:(kt + 1) * 128,
                                          mg * 256:(mg + 1) * 256])
                            for mi in range(2):
                                for th in range(2):
                                    nc.tensor.matmul(
                                        pss[mi * 2 + th][:],
                                        w2[:, mi * 128:(mi + 1) * 128],
                                        attnT[kt][:, th * 512:(th + 1) * 512],
                                        start=(kt == 0), stop=(kt == NKT - 1))
                        for mi in range(2):
                            m = mg * 2 + mi
                            xot = xos_p.tile([128, TL], f32r, tag="x",
                                             name="xot")
                            nc.sync.dma_start(
                                xot[:], xT_own.ap()[m * 128:(m + 1) * 128, :])
                            for th in range(2):
                                nc.vector.scalar_tensor_tensor(
                                    xres[m][:, th * 512:(th + 1) * 512],
                                    pss[mi * 2 + th][:],
                                    bap_sb[:, m:m + 1],
                                    xot[:, th * 512:(th + 1) * 512],
                                    ALU.add, ALU.add)

            # ============== LN1 -> x1 ; FFN ; LN2 -> out ==============
            with tc.tile_pool(name="x1", bufs=1) as x1_p:
                x1 = [x1_p.tile([128, TL], f32r, tag=f"x1{kt}",
                                name=f"x1t{kt}") for kt in range(NKT)]

                def layernorm(src, emit_out, psln, tmp_p):
                    # src: list of NKT [128, TL] f32r tiles (feature-major)
                    ps_sum = psln.tile([128, TL], f32, tag="su", name="ps_sum")
                    ps_sq = psln.tile([128, TL], f32, tag="sq", name="ps_sq")
                    for kt in range(NKT):
                        sq = tmp_p.tile([128, TL], f32r, tag="sqt", name="sq")
                        nc.scalar.activation(sq[:], src[kt][:], AF.Square)
                        for th in range(2):
                            sl = slice(th * 512, (th + 1) * 512)
                            nc.tensor.matmul(
                                ps_sum[:, sl], ones_r[:], src[kt][:, sl],
                                start=(kt == 0), stop=(kt == NKT - 1))
                            nc.tensor.matmul(
                                ps_sq[:, sl], ones_r[:], sq[:, sl],
                                start=(kt == 0), stop=(kt == NKT - 1))
                    mu = tmp_p.tile([128, TL], f32, tag="mu", name="mu")
                    nc.vector.tensor_scalar_mul(mu[:], ps_sum[:], 1.0 / C)
                    msq = tmp_p.tile([128, TL], f32, tag="msq", name="msq")
                    nc.vector.tensor_scalar_mul(msq[:], ps_sq[:], 1.0 / C)
                    mu2 = tmp_p.tile([128, TL], f32, tag="mu2", name="mu2")
                    nc.vector.tensor_mul(mu2[:], mu[:], mu[:])
                    var = tmp_p.tile([128, TL], f32, tag="var", name="var")
                    nc.vector.tensor_sub(var[:], msq[:], mu2[:])
                    sd = tmp_p.tile([128, TL], f32, tag="sd", name="sd")
                    nc.scalar.activation(sd[:], var[:], AF.Sqrt, bias=eps_sb[:])
                    rstd = tmp_p.tile([128, TL], f32, tag="rstd", name="rstd")
                    nc.vector.reciprocal(rstd[:], sd[:])
                    for kt in range(NKT):
                        t1 = tmp_p.tile([128, TL], f32, tag="t1", name="t1")
                        nc.vector.tensor_sub(t1[:], src[kt][:], mu[:])
                        t2 = tmp_p.tile([128, TL], f32, tag="t2", name="t2")
                        nc.vector.tensor_mul(t2[:], t1[:], rstd[:])
                        emit_out(kt, t2)

                # ---- LN1 ----
                with tc.tile_pool(name="lt1", bufs=1) as lt1_p, \
                     tc.tile_pool(name="psl1", bufs=1, space="PSUM") as psl1:
                    def emit_x1(kt, t2):
                        nc.scalar.activation(
                            x1[kt][:], t2[:], AF.Identity,
                            bias=be1_sb[:, kt:kt + 1],
                            scale=g1_sb[:, kt:kt + 1])

                    layernorm(xres, emit_x1, psl1, lt1_p)

                # ---- FFN ----
                with tc.tile_pool(name="hb", bufs=1) as hb_p, \
                     tc.tile_pool(name="wf", bufs=4) as wf_p, \
                     tc.tile_pool(name="wfp", bufs=6) as wfp_p, \
                     tc.tile_pool(name="ps_h", bufs=1, space="PSUM") as ps_h_p, \
                     tc.tile_pool(name="ps_y", bufs=1, space="PSUM") as ps_y_p:
                    for th in range(2):
                        hs = [hb_p.tile([128, 512], bf16, tag=f"h{mh}",
                                        name=f"hs{mh}") for mh in range(NHT)]
                        # FF1: h = gelu(W_ff.T @ x1 + b_ff), half of T
                        for mtg in range(NHT // 4):
                            pss = [ps_h_p.tile([128, 512], f32, tag=f"h{i}",
                                               name=f"psh{i}")
                                   for i in range(4)]
                            for kt in range(NKT):
                                wf = wf_p.tile([128, 512], f32r, tag="w")
                                nc.sync.dma_start(
                                    wf[:],
                                    W_ff.ap()[kt * 128:(kt + 1) * 128,
                                              mtg * 512:(mtg + 1) * 512])
                                for mt in range(4):
                                    nc.tensor.matmul(
                                        pss[mt][:],
                                        wf[:, mt * 128:(mt + 1) * 128],
                                        x1[kt][:, th * 512:(th + 1) * 512],
                                        start=(kt == 0), stop=(kt == NKT - 1))
                            for mt in range(4):
                                mh = mtg * 4 + mt
                                nc.scalar.activation(
                                    hs[mh][:], pss[mt][:], AF.Gelu,
                                    bias=bff_sb[:, mh:mh + 1])
                        # FF2: x2 = x1 + W_ffp.T @ h + b_ffp (into x1)
                        for mg in range(4):
                            pss = [ps_y_p.tile([128, 512], f32, tag=f"y{i}",
                                               name=f"psy{i}")
                                   for i in range(4)]
                            for kt in range(NHT):
                                wfp = wfp_p.tile([128, 512], bf16, tag="w")
                                nc.sync.dma_start(
                                    wfp[:],
                                    W_ffp.ap()[kt * 128:(kt + 1) * 128,
                                               mg * 512:(mg + 1) * 512])
                                for mi in range(4):
                                    nc.tensor.matmul(
                                        pss[mi][:],
                                        wfp[:, mi * 128:(mi + 1) * 128],
                                        hs[kt][:],
                                        start=(kt == 0), stop=(kt == NHT - 1))
                            for mi in range(4):
                                m = mg * 4 + mi
                                sl = slice(th * 512, (th + 1) * 512)
                                nc.vector.scalar_tensor_tensor(
                                    x1[m][:, sl], pss[mi][:],
                                    bffp_sb[:, m:m + 1], x1[m][:, sl],
                                    ALU.add, ALU.add)

                # ---- LN2 -> output ----
                with tc.tile_pool(name="lt2", bufs=1) as lt2_p, \
                     tc.tile_pool(name="ob", bufs=3) as ob_p, \
                     tc.tile_pool(name="psl2", bufs=1, space="PSUM") as psl2:
                    def emit_out(kt, t2):
                        ot = ob_p.tile([128, TL], f32, tag="o", name="ot")
                        nc.scalar.activation(
                            ot[:], t2[:], AF.Identity,
                            bias=be2_sb[:, kt:kt + 1],
                            scale=g2_sb[:, kt:kt + 1])
                        nc.sync.dma_start(
                            outT.ap()[kt * 128:(kt + 1) * 128, :], ot[:])

                    layernorm(x1, emit_out, psl2, lt2_p)

    nc.compile()
    return nc


def _get_nc():
    if "nc" not in _CACHE:
        _CACHE["nc"] = build_nc()
    return _CACHE["nc"]


def _prepare_in_maps(inputs):
    x = np.asarray(inputs["x"], dtype=np.float32)
    W_qkv = np.asarray(inputs["W_qkv"], dtype=np.float32)
    b_qkv = np.asarray(inputs["b_qkv"], dtype=np.float32)
    W_ap = np.asarray(inputs["W_attn_proj"], dtype=np.float32)
    b_ap = np.asarray(inputs["b_attn_proj"], dtype=np.float32)
    ln1_g = np.asarray(inputs["ln1_g"], dtype=np.float32)
    ln1_b = np.asarray(inputs["ln1_b"], dtype=np.float32)
    W_ff = np.asarray(inputs["W_ff"], dtype=np.float32)
    b_ff = np.asarray(inputs["b_ff"], dtype=np.float32)
    W_ffp = np.asarray(inputs["W_ff_proj"], dtype=np.float32)
    b_ffp = np.asarray(inputs["b_ff_proj"], dtype=np.float32)
    ln2_g = np.asarray(inputs["ln2_g"], dtype=np.float32)
    ln2_b = np.asarray(inputs["ln2_b"], dtype=np.float32)

    import ml_dtypes
    W_ffp_bf = W_ffp.astype(ml_dtypes.bfloat16)

    b_qs = np.ascontiguousarray(b_qkv[:C] * SCALE)
    b_kk = np.ascontiguousarray(b_qkv[C:2 * C])
    b_vv = np.ascontiguousarray(b_qkv[2 * C:])

    iota_np = np.broadcast_to(np.arange(TL, dtype=np.float32),
                              (128, TL)).copy()
    p_idx = np.arange(128, dtype=np.float32)[:, None]
    kt_idx = np.arange(NKT, dtype=np.float32)[None, :] * 128
    thrs = [np.ascontiguousarray(p_idx + kt_idx - hh * TL)
            for hh in range(2)]

    xT_fulls = [np.ascontiguousarray(x[b].T) for b in range(B)]

    common = dict(W_qkv=W_qkv, b_qs=b_qs, b_k=b_kk, b_v=b_vv, W_ap=W_ap,
                  b_ap=b_ap, ln1_g=ln1_g, ln1_b=ln1_b, W_ff=W_ff, b_ff=b_ff,
                  W_ffp=W_ffp_bf, b_ffp=b_ffp, ln2_g=ln2_g, ln2_b=ln2_b)
    in_maps = []
    for c in range(N_CORES):
        b, hh = c // 2, c % 2
        m = dict(common)
        m["xT_full"] = xT_fulls[b]
        m["xT_own"] = np.ascontiguousarray(
            xT_fulls[b][:, hh * TL:(hh + 1) * TL])
        m["iota_q"] = iota_np
        m["thr_all"] = thrs[hh]
        in_maps.append(m)
    return in_maps


def run_kernel(inputs, trace=False):
    nc = _get_nc()
    in_maps = _prepare_in_maps(inputs)
    res = bass_utils.run_bass_kernel_spmd(
        nc, in_maps, core_ids=list(range(N_CORES)), trace=trace)
    out = np.empty((B, T, C), dtype=np.float32)
    for c in range(N_CORES):
        b, hh = c // 2, c % 2
        out[b, hh * TL:(hh + 1) * TL, :] = res.results[c]["outT"].T
    return out, res


def kernel(**inputs) -> np.ndarray:
    out, _ = run_kernel(inputs, trace=False)
    return out


# revision 10
# speedup vs baseline: 1.1341x; 1.0903x over previous
"""Trainium2 Bass kernel for a dense transformer block (attention + GELU MLP).

Strategy (8 cores, SPMD single program):
  - Token/data parallel: core c handles batch b=c//2, token half hh=c%2
    (1024 of 2048 tokens). All weights replicated.
  - K/V are computed for the FULL batch on both cores of a pair
    (duplicated compute, zero communication), Q only for own tokens.
  - Everything on-device is kept feature-major (activations stored
    transposed, [features, tokens]) so every matmul consumes weights in
    their natural [C_in, C_out] layout and no transposes are needed:
        y^T = lhsT.T @ rhs  with lhsT = W, rhs = x^T.
  - Attention: scoresT[k, q] = K_h @ Q_h^T via lhsT=K_h^T slices;
    softmax without max-subtraction (logits ~ N(0,1), fp32 exp is safe);
    causal mask is a multiplicative 0/1 input applied post-exp;
    attn^T = V_h^T @ expT via lhsT=V_h (token-major V); denominators via
    an all-ones stationary matmul accumulated per k-tile on the PE
    (which is otherwise underutilized during attention, and whose
    redundant 128-row output doubles as a partition broadcast).
  - LayerNorm over the partition (feature) axis: sums via ones-matmuls,
    E[x^2] via ACT Square, normalize on DVE, final scale/bias on ACT.
  - Matmuls run in float32r (full PE rate at N>=512, ~1e-4 rounding);
    the FF2 h-path uses bf16 to halve its SBUF/DMA footprint.
  - Big activations are allocated as per-128-row tiles (not one big
    tile) so cross-phase dependencies stay fine-grained.
"""

import sys

sys.path.insert(0, "/opt/trn_rl_repo")

import math
import numpy as np

import concourse.bass as bass
import concourse.bacc as bacc
import concourse.mybir as mybir
import concourse.tile as tile
from concourse import bass_utils

N_CORES = 8
B, T, C = 4, 2048, 2048
NH, D = 16, 128
HF = 4 * C
TL = T // 2        # own tokens per core
TB = T             # batch tokens (K/V extent)
NKT = C // 128     # 16 feature tiles
NHT = HF // 128    # 64 FFN hidden tiles
LN_EPS = 1e-5
SCALE = 1.0 / math.sqrt(D)

f32 = mybir.dt.float32
f32r = mybir.dt.float32r
bf16 = mybir.dt.bfloat16
AF = mybir.ActivationFunctionType
ALU = mybir.AluOpType

_CACHE = {}


def _bias_ap(t, n):
    # [n*128] dram tensor viewed as [128, n] (partition = within-tile index)
    return bass.AP(t, 0, [[1, 128], [128, n]])


def build_nc():
    nc = bacc.Bacc("TRN2", target_bir_lowering=False, debug=False,
                   num_devices=N_CORES)

    # ---- I/O ----
    xT_full = nc.dram_tensor("xT_full", [C, TB], f32r, kind="ExternalInput")
    iota_q = nc.dram_tensor("iota_q", [128, TL], f32, kind="ExternalInput")
    thr_all = nc.dram_tensor("thr_all", [128, NKT], f32, kind="ExternalInput")
    W_qkv = nc.dram_tensor("W_qkv", [C, 3 * C], f32r, kind="ExternalInput")
    b_qs = nc.dram_tensor("b_qs", [C], f32, kind="ExternalInput")
    b_k = nc.dram_tensor("b_k", [C], f32, kind="ExternalInput")
    b_v = nc.dram_tensor("b_v", [C], f32, kind="ExternalInput")
    W_ap = nc.dram_tensor("W_ap", [C, C], f32r, kind="ExternalInput")
    b_ap = nc.dram_tensor("b_ap", [C], f32, kind="ExternalInput")
    ln1_g = nc.dram_tensor("ln1_g", [C], f32, kind="ExternalInput")
    ln1_b = nc.dram_tensor("ln1_b", [C], f32, kind="ExternalInput")
    W_ff = nc.dram_tensor("W_ff", [C, HF], f32r, kind="ExternalInput")
    b_ff = nc.dram_tensor("b_ff", [HF], f32, kind="ExternalInput")
    W_ffp = nc.dram_tensor("W_ffp", [HF, C], bf16, kind="ExternalInput")
    b_ffp = nc.dram_tensor("b_ffp", [C], f32, kind="ExternalInput")
    ln2_g = nc.dram_tensor("ln2_g", [C], f32, kind="ExternalInput")
    ln2_b = nc.dram_tensor("ln2_b", [C], f32, kind="ExternalInput")
    outT = nc.dram_tensor("outT", [C, TL], f32, kind="ExternalOutput")

    with tile.TileContext(nc) as tc:
        with tc.tile_pool(name="dram", bufs=1, space="DRAM") as dram, \
             tc.tile_pool(name="consts", bufs=1) as cp:
            qT = dram.tile([C, TL], f32r)
            kT = dram.tile([C, TB], f32r)
            vTok = dram.tile([TB, C], f32r)

            # constants: biases as [128, n] tiles + all-ones stationary
            bqs_sb = cp.tile([128, NKT], f32)
            nc.sync.dma_start(bqs_sb[:], _bias_ap(b_qs, NKT))
            bk_sb = cp.tile([128, NKT], f32)
            nc.sync.dma_start(bk_sb[:], _bias_ap(b_k, NKT))
            bv_sb = cp.tile([128, NKT], f32)
            nc.sync.dma_start(bv_sb[:], _bias_ap(b_v, NKT))
            bap_sb = cp.tile([128, NKT], f32)
            nc.sync.dma_start(bap_sb[:], _bias_ap(b_ap, NKT))
            g1_sb = cp.tile([128, NKT], f32)
            nc.sync.dma_start(g1_sb[:], _bias_ap(ln1_g, NKT))
            be1_sb = cp.tile([128, NKT], f32)
            nc.sync.dma_start(be1_sb[:], _bias_ap(ln1_b, NKT))
            bff_sb = cp.tile([128, NHT], f32)
            nc.sync.dma_start(bff_sb[:], _bias_ap(b_ff, NHT))
            bffp_sb = cp.tile([128, NKT], f32)
            nc.sync.dma_start(bffp_sb[:], _bias_ap(b_ffp, NKT))
            g2_sb = cp.tile([128, NKT], f32)
            nc.sync.dma_start(g2_sb[:], _bias_ap(ln2_g, NKT))
            be2_sb = cp.tile([128, NKT], f32)
            nc.sync.dma_start(be2_sb[:], _bias_ap(ln2_b, NKT))
            ones_f = cp.tile([128, 128], f32)
            nc.vector.memset(ones_f[:], 1.0)
            ones_r = cp.tile([128, 128], f32r)
            nc.scalar.copy(ones_r[:], ones_f[:])
            eps_sb = cp.tile([128, 1], f32)
            nc.vector.memset(eps_sb[:], LN_EPS)
            iota_sb = cp.tile([128, TL], f32)
            nc.sync.dma_start(iota_sb[:], iota_q.ap())
            thr_sb = cp.tile([128, NKT], f32)
            nc.sync.dma_start(thr_sb[:], thr_all.ap())

            # ====== Phase Q, V, K over permuted (own-first) token order ======
            with tc.tile_pool(name="xf", bufs=1) as xf_p, \
                 tc.tile_pool(name="wk", bufs=6) as wk_p, \
                 tc.tile_pool(name="kv_e", bufs=6) as kve_p, \
                 tc.tile_pool(name="qe", bufs=4) as qe_p, \
                 tc.tile_pool(name="psA", bufs=2, space="PSUM") as psA:
                xf = []
                for kt in range(NKT):
                    xft = xf_p.tile([128, TB], f32r, tag=f"xf{kt}",
                                    name=f"xf{kt}")
                    nc.sync.dma_start(
                        xft[:], xT_full.ap()[kt * 128:(kt + 1) * 128, :])
                    xf.append(xft)
                # Q (own half = first TL permuted tokens) -> qT
                for mg in range(8):
                    pss = [psA.tile([128, 512], f32, tag=f"a{i}",
                                    name=f"psq{i}") for i in range(4)]
                    for kt in range(NKT):
                        w2 = wk_p.tile([128, 256], f32r, tag="wq")
                        nc.sync.dma_start(
                            w2[:],
                            W_qkv.ap()[kt * 128:(kt + 1) * 128,
                                       mg * 256:(mg + 1) * 256])
                        for mi in range(2):
                            for th in range(2):
                                nc.tensor.matmul(
                                    pss[mi * 2 + th][:],
                                    w2[:, mi * 128:(mi + 1) * 128],
                                    xf[kt][:, th * 512:(th + 1) * 512],
                                    start=(kt == 0), stop=(kt == NKT - 1))
                    for mi in range(2):
                        m = mg * 2 + mi
                        for th in range(2):
                            ev = qe_p.tile([128, 512], f32r, tag="e")
                            nc.vector.tensor_scalar(
                                ev[:], pss[mi * 2 + th][:], SCALE,
                                bqs_sb[:, m:m + 1], ALU.mult, ALU.add)
                            nc.sync.dma_start(
                                qT[m * 128:(m + 1) * 128,
                                   th * 512:(th + 1) * 512], ev[:])
                # V token-major [2048, C] (permuted order)
                for nv in range(4):
                    for mtg in range(2):
                        pss = [psA.tile([128, 512], f32, tag=f"a{i % 4}",
                                        name=f"pskv{i}") for i in range(8)]
                        for kt in range(NKT):
                            wv = wk_p.tile([128, 512], f32r, tag="wv")
                            nc.sync.dma_start(
                                wv[:],
                                W_qkv.ap()[kt * 128:(kt + 1) * 128,
                                           2 * C + nv * 512:
                                           2 * C + (nv + 1) * 512])
                            for mt in range(8):
                                tt = mtg * 8 + mt
                                nc.tensor.matmul(
                                    pss[mt][:],
                                    xf[kt][:, tt * 128:(tt + 1) * 128],
                                    wv[:],
                                    start=(kt == 0), stop=(kt == NKT - 1))
                        for mt in range(8):
                            tt = mtg * 8 + mt
                            ev = kve_p.tile([128, 512], f32r, tag="e")
                            nc.vector.tensor_copy(ev[:], pss[mt][:])
                            nc.sync.dma_start(
                                vTok[tt * 128:(tt + 1) * 128,
                                     nv * 512:(nv + 1) * 512], ev[:])
                # K^T [C, 2048] (permuted order)
                for m in range(NKT):
                    pss = [psA.tile([128, 512], f32, tag=f"a{i}",
                                    name=f"psk{i}") for i in range(4)]
                    for kt in range(NKT):
                        w1 = wk_p.tile([128, 128], f32r, tag="w")
                        nc.sync.dma_start(
                            w1[:],
                            W_qkv.ap()[kt * 128:(kt + 1) * 128,
                                       C + m * 128:C + (m + 1) * 128])
                        for tcn in range(4):
                            nc.tensor.matmul(
                                pss[tcn][:], w1[:],
                                xf[kt][:, tcn * 512:(tcn + 1) * 512],
                                start=(kt == 0), stop=(kt == NKT - 1))
                    for tcn in range(4):
                        ev = kve_p.tile([128, 512], f32r, tag="e")
                        nc.vector.tensor_scalar_add(
                            ev[:], pss[tcn][:], bk_sb[:, m:m + 1])
                        nc.sync.dma_start(
                            kT[m * 128:(m + 1) * 128,
                               tcn * 512:(tcn + 1) * 512], ev[:])

            # =================== Attention + projection ===================
            # xres pool opened first so it survives into LN1 (stack alloc).
            with tc.tile_pool(name="xres", bufs=1) as xr_p, \
                 tc.tile_pool(name="attnT", bufs=1) as at_p:
                xres = [xr_p.tile([128, TL], f32r, tag=f"xr{kt}",
                                  name=f"xres{kt}") for kt in range(NKT)]
                attnT = [at_p.tile([128, TL], f32r, tag=f"at{h}",
                                   name=f"attnT{h}") for h in range(NH)]
                with tc.tile_pool(name="kv", bufs=1) as kv_p, \
                     tc.tile_pool(name="qh", bufs=2) as qh_p, \
                     tc.tile_pool(name="et", bufs=2) as et_p, \
                     tc.tile_pool(name="ex", bufs=2) as ex_p, \
                     tc.tile_pool(name="nrm", bufs=1) as nrm_p, \
                     tc.tile_pool(name="ps_s", bufs=2, space="PSUM") as ps_s_p, \
                     tc.tile_pool(name="ps_av", bufs=1, space="PSUM") as ps_av_p, \
                     tc.tile_pool(name="ps_d", bufs=1, space="PSUM") as ps_d_p:
                    for h in range(NH):
                        kc = []
                        vc = []
                        for cq in range(4):
                            kct = kv_p.tile([128, 512], f32r, tag=f"kc{cq}",
                                            name=f"kc{cq}")
                            nc.sync.dma_start(
                                kct[:], kT[h * 128:(h + 1) * 128,
                                           cq * 512:(cq + 1) * 512])
                            kc.append(kct)
                            vct = kv_p.tile([128, 512], f32r, tag=f"vc{cq}",
                                            name=f"vc{cq}")
                            nc.sync.dma_start(
                                vct[:],
                                bass.AP(vTok.tensor,
                                        h * 128 + cq * 4 * 128 * C,
                                        [[C, 128], [128 * C, 4], [1, 128]]))
                            vc.append(vct)
                        q_sb = qh_p.tile([128, TL], f32r, tag="q")
                        nc.sync.dma_start(
                            q_sb[:], qT[h * 128:(h + 1) * 128, :])
                        ps_av = ps_av_p.tile([128, TL], f32, tag="av",
                                             name="ps_av")
                        ps_d = ps_d_p.tile([128, TL], f32, tag="dn",
                                           name="ps_d")
                        for kt in range(NKT):
                            ps_s = ps_s_p.tile([128, TL], f32, tag="s",
                                               name="ps_s")
                            for th in range(2):
                                nc.tensor.matmul(
                                    ps_s[:, th * 512:(th + 1) * 512],
                                    kc[kt // 4][:, (kt % 4) * 128:
                                                (kt % 4 + 1) * 128],
                                    q_sb[:, th * 512:(th + 1) * 512],
                                    start=True, stop=True)
                            et = et_p.tile([128, TL], f32, tag="et",
                                           name="et")
                            nc.scalar.activation(et[:], ps_s[:], AF.Exp)
                            e_kt = ex_p.tile([128, TL], f32r, tag="ex",
                                             name="e_kt")
                            nc.vector.scalar_tensor_tensor(
                                e_kt[:], iota_sb[:], thr_sb[:, kt:kt + 1],
                                et[:], ALU.is_ge, ALU.mult)
                            for th in range(2):
                                sl = slice(th * 512, (th + 1) * 512)
                                nc.tensor.matmul(
                                    ps_av[:, sl],
                                    vc[kt // 4][:, (kt % 4) * 128:
                                                (kt % 4 + 1) * 128],
                                    e_kt[:, sl],
                                    start=(kt == 0), stop=(kt == NKT - 1))
                                nc.tensor.matmul(
                                    ps_d[:, sl], ones_r[:], e_kt[:, sl],
                                    start=(kt == 0), stop=(kt == NKT - 1))
                        rec = nrm_p.tile([128, TL], f32, tag="rec",
                                         name="rec")
                        nc.vector.reciprocal(rec[:], ps_d[:])
                        avn = nrm_p.tile([128, TL], f32, tag="avn",
                                         name="avn")
                        nc.vector.tensor_mul(avn[:], ps_av[:], rec[:])
                        nc.scalar.activation(
                            attnT[h][:], avn[:],
                            AF.Identity, bias=bv_sb[:, h:h + 1])

                # ---- attention projection + residual -> xres (SBUF) ----
                with tc.tile_pool(name="wp", bufs=6) as wp_p, \
                     tc.tile_pool(name="xos", bufs=3) as xos_p, \
                     tc.tile_pool(name="psp", bufs=2, space="PSUM") as psp:
                    for mg in range(8):
                        pss = [psp.tile([128, 512], f32, tag=f"p{i}",
                                        name=f"psp{i}") for i in range(4)]
                        for kt in range(NKT):
                            w2 = wp_p.tile([128, 256], f32r, tag="w")
                            nc.sync.dma_start(
                                w2[:],
                                W_ap.ap()[kt * 128:(kt + 1) * 128,
                                          mg * 256:(mg + 1) * 256])
                            for mi in range(2):
                                for th in range(2):
                                    nc.tensor.matmul(
                                        pss[mi * 2 + th][:],
                                        w2[:, mi * 128:(mi + 1) * 128],
                                        attnT[kt][:, th * 512:(th + 1) * 512],
                                        start=(kt == 0), stop=(kt == NKT - 1))
                        for mi in range(2):
                            m = mg * 2 + mi
                            xot = xos_p.tile([128, TL], f32r, tag="x",
                                             name="xot")
                            nc.sync.dma_start(
                                xot[:], xT_full.ap()[m * 128:(m + 1) * 128,
                                                     0:TL])
                            for th in range(2):
                                nc.vector.scalar_tensor_tensor(
                                    xres[m][:, th * 512:(th + 1) * 512],
                                    pss[mi * 2 + th][:],
                                    bap_sb[:, m:m + 1],
                                    xot[:, th * 512:(th + 1) * 512],
                                    ALU.add, ALU.add)

            # ============== LN1 -> x1 ; FFN ; LN2 -> out ==============
            with tc.tile_pool(name="x1", bufs=1) as x1_p:
                x1 = [x1_p.tile([128, TL], f32r, tag=f"x1{kt}",
                                name=f"x1t{kt}") for kt in range(NKT)]

                def layernorm(src, emit_out, psln, tmp_p):
                    # src: list of NKT [128, TL] f32r tiles (feature-major)
                    ps_sum = psln.tile([128, TL], f32, tag="su", name="ps_sum")
                    ps_sq = psln.tile([128, TL], f32, tag="sq", name="ps_sq")
                    for kt in range(NKT):
                        sq = tmp_p.tile([128, TL], f32r, tag="sqt", name="sq")
                        nc.scalar.activation(sq[:], src[kt][:], AF.Square)
                        for th in range(2):
                            sl = slice(th * 512, (th + 1) * 512)
                            nc.tensor.matmul(
                                ps_sum[:, sl], ones_r[:], src[kt][:, sl],
                                start=(kt == 0), stop=(kt == NKT - 1))
                            nc.tensor.matmul(
                                ps_sq[:, sl], ones_r[:], sq[:, sl],
                                start=(kt == 0), stop=(kt == NKT - 1))
                    mu = tmp_p.tile([128, TL], f32, tag="mu", name="mu")
                    nc.vector.tensor_scalar_mul(mu[:], ps_sum[:], 1.0 / C)
                    msq = tmp_p.tile([128, TL], f32, tag="msq", name="msq")
                    nc.vector.tensor_scalar_mul(msq[:], ps_sq[:], 1.0 / C)
                    mu2 = tmp_p.tile([128, TL], f32, tag="mu2", name="mu2")
                    nc.vector.tensor_mul(mu2[:], mu[:], mu[:])
                    var = tmp_p.tile([128, TL], f32, tag="var", name="var")
                    nc.vector.tensor_sub(var[:], msq[:], mu2[:])
                    sd = tmp_p.tile([128, TL], f32, tag="sd", name="sd")
                    nc.scalar.activation(sd[:], var[:], AF.Sqrt, bias=eps_sb[:])
                    rstd = tmp_p.tile([128, TL], f32, tag="rstd", name="rstd")
                    nc.vector.reciprocal(rstd[:], sd[:])
                    for kt in range(NKT):
                        t1 = tmp_p.tile([128, TL], f32, tag="t1", name="t1")
                        nc.vector.tensor_sub(t1[:], src[kt][:], mu[:])
                        t2 = tmp_p.tile([128, TL], f32, tag="t2", name="t2")
                        nc.vector.tensor_mul(t2[:], t1[:], rstd[:])
                        emit_out(kt, t2)

                # ---- LN1 ----
                with tc.tile_pool(name="lt1", bufs=1) as lt1_p, \
                     tc.tile_pool(name="psl1", bufs=1, space="PSUM") as psl1:
                    def emit_x1(kt, t2):
                        nc.scalar.activation(
                            x1[kt][:], t2[:], AF.Identity,
                            bias=be1_sb[:, kt:kt + 1],
                            scale=g1_sb[:, kt:kt + 1])

                    layernorm(xres, emit_x1, psl1, lt1_p)

                # ---- FFN ----
                with tc.tile_pool(name="hb", bufs=1) as hb_p, \
                     tc.tile_pool(name="wf", bufs=4) as wf_p, \
                     tc.tile_pool(name="wfp", bufs=6) as wfp_p, \
                     tc.tile_pool(name="ps_h", bufs=1, space="PSUM") as ps_h_p, \
                     tc.tile_pool(name="ps_y", bufs=1, space="PSUM") as ps_y_p:
                    for th in range(2):
                        hs = [hb_p.tile([128, 512], bf16, tag=f"h{mh}",
                                        name=f"hs{mh}") for mh in range(NHT)]
                        # FF1: h = gelu(W_ff.T @ x1 + b_ff), half of T
                        for mtg in range(NHT // 4):
                            pss = [ps_h_p.tile([128, 512], f32, tag=f"h{i}",
                                               name=f"psh{i}")
                                   for i in range(4)]
                            for kt in range(NKT):
                                wf = wf_p.tile([128, 512], f32r, tag="w")
                                nc.sync.dma_start(
                                    wf[:],
                                    W_ff.ap()[kt * 128:(kt + 1) * 128,
                                              mtg * 512:(mtg + 1) * 512])
                                for mt in range(4):
                                    nc.tensor.matmul(
                                        pss[mt][:],
                                        wf[:, mt * 128:(mt + 1) * 128],
                                        x1[kt][:, th * 512:(th + 1) * 512],
                                        start=(kt == 0), stop=(kt == NKT - 1))
                            for mt in range(4):
                                mh = mtg * 4 + mt
                                nc.scalar.activation(
                                    hs[mh][:], pss[mt][:], AF.Gelu,
                                    bias=bff_sb[:, mh:mh + 1])
                        # FF2: x2 = x1 + W_ffp.T @ h + b_ffp (into x1)
                        for mg in range(4):
                            pss = [ps_y_p.tile([128, 512], f32, tag=f"y{i}",
                                               name=f"psy{i}")
                                   for i in range(4)]
                            for kt in range(NHT):
                                wfp = wfp_p.tile([128, 512], bf16, tag="w")
                                nc.sync.dma_start(
                                    wfp[:],
                                    W_ffp.ap()[kt * 128:(kt + 1) * 128,
                                               mg * 512:(mg + 1) * 512])
                                for mi in range(4):
                                    nc.tensor.matmul(
                                        pss[mi][:],
                                        wfp[:, mi * 128:(mi + 1) * 128],
                                        hs[kt][:],
                                        start=(kt == 0), stop=(kt == NHT - 1))
                            for mi in range(4):
                                m = mg * 4 + mi
                                sl = slice(th * 512, (th + 1) * 512)
                                nc.vector.scalar_tensor_tensor(
                                    x1[m][:, sl], pss[mi][:],
                                    bffp_sb[:, m:m + 1], x1[m][:, sl],
                                    ALU.add, ALU.add)

                # ---- LN2 -> output ----
                with tc.tile_pool(name="lt2", bufs=1) as lt2_p, \
                     tc.tile_pool(name="ob", bufs=3) as ob_p, \
                     tc.tile_pool(name="psl2", bufs=1, space="PSUM") as psl2:
                    def emit_out(kt, t2):
                        ot = ob_p.tile([128, TL], f32, tag="o", name="ot")
                        nc.scalar.activation(
                            ot[:], t2[:], AF.Identity,
                            bias=be2_sb[:, kt:kt + 1],
                            scale=g2_sb[:, kt:kt + 1])
                        nc.sync.dma_start(
                            outT.ap()[kt * 128:(kt + 1) * 128, :], ot[:])

                    layernorm(x1, emit_out, psl2, lt2_p)

    nc.compile()
    return nc


def _get_nc():
    if "nc" not in _CACHE:
        _CACHE["nc"] = build_nc()
    return _CACHE["nc"]


def _prepare_in_maps(inputs):
    x = np.asarray(inputs["x"], dtype=np.float32)
    W_qkv = np.asarray(inputs["W_qkv"], dtype=np.float32)
    b_qkv = np.asarray(inputs["b_qkv"], dtype=np.float32)
    W_ap = np.asarray(inputs["W_attn_proj"], dtype=np.float32)
    b_ap = np.asarray(inputs["b_attn_proj"], dtype=np.float32)
    ln1_g = np.asarray(inputs["ln1_g"], dtype=np.float32)
    ln1_b = np.asarray(inputs["ln1_b"], dtype=np.float32)
    W_ff = np.asarray(inputs["W_ff"], dtype=np.float32)
    b_ff = np.asarray(inputs["b_ff"], dtype=np.float32)
    W_ffp = np.asarray(inputs["W_ff_proj"], dtype=np.float32)
    b_ffp = np.asarray(inputs["b_ff_proj"], dtype=np.float32)
    ln2_g = np.asarray(inputs["ln2_g"], dtype=np.float32)
    ln2_b = np.asarray(inputs["ln2_b"], dtype=np.float32)

    import ml_dtypes
    W_ffp_bf = W_ffp.astype(ml_dtypes.bfloat16)

    b_qs = np.ascontiguousarray(b_qkv[:C] * SCALE)
    b_kk = np.ascontiguousarray(b_qkv[C:2 * C])
    b_vv = np.ascontiguousarray(b_qkv[2 * C:])

    iota_np = np.broadcast_to(np.arange(TL, dtype=np.float32),
                              (128, TL)).copy()
    p_idx = np.arange(128, dtype=np.float32)[:, None]
    kt_idx = np.arange(NKT, dtype=np.float32)[None, :]
    # permuted (own-first) k order: kt<8 -> own blocks, kt>=8 -> other half
    thrs = []
    for hh in range(2):
        thr = np.empty((128, NKT), dtype=np.float32)
        thr[:, :8] = p_idx + kt_idx[:, :8] * 128
        thr[:, 8:] = p_idx + (kt_idx[:, 8:] - 8) * 128 + (1 - 2 * hh) * TL
        thrs.append(np.ascontiguousarray(thr))

    xT_b = [x[b].T for b in range(B)]

    common = dict(W_qkv=W_qkv, b_qs=b_qs, b_k=b_kk, b_v=b_vv, W_ap=W_ap,
                  b_ap=b_ap, ln1_g=ln1_g, ln1_b=ln1_b, W_ff=W_ff, b_ff=b_ff,
                  W_ffp=W_ffp_bf, b_ffp=b_ffp, ln2_g=ln2_g, ln2_b=ln2_b)
    in_maps = []
    for c in range(N_CORES):
        b, hh = c // 2, c % 2
        m = dict(common)
        m["xT_full"] = np.ascontiguousarray(np.concatenate(
            [xT_b[b][:, hh * TL:(hh + 1) * TL],
             xT_b[b][:, (1 - hh) * TL:(2 - hh) * TL]], axis=1))
        m["iota_q"] = iota_np
        m["thr_all"] = thrs[hh]
        in_maps.append(m)
    return in_maps


def run_kernel(inputs, trace=False):
    nc = _get_nc()
    in_maps = _prepare_in_maps(inputs)
    res = bass_utils.run_bass_kernel_spmd(
        nc, in_maps, core_ids=list(range(N_CORES)), trace=trace)
    out = np.empty((B, T, C), dtype=np.float32)
    for c in range(N_CORES):
        b, hh = c // 2, c % 2
        out[b, hh * TL:(hh + 1) * TL, :] = res.results[c]["outT"].T
    return out, res


def kernel(**inputs) -> np.ndarray:
    out, _ = run_kernel(inputs, trace=False)
    return out
